# revision 5
# baseline (speedup 1.0000x reference)
"""Trainium2 Bass kernel for nn_BayesianFlowNetworkDiscretised (v3).

Per (b, d) position:
    MLP: h_j = gelu_tanh(W1[0,j]*mu + t*W1[1,j] + b1[j]);  (mu_eps, ln_sig) = h@W2 + b2
    mu_x = mu/gamma - var_scale*mu_eps
    sigma = max(var_scale*exp(ln_sig), 0.02)
    out_k = Phi((e_{k+1}-mu_x)/sigma) - Phi((e_k-mu_x)/sigma),  e_i = i/8 - 1

Structure:
  - mu is partition-replicated 2x by DMA straight from DRAM (stride-2
    partition APs) into two expanded blocks; the per-unit W1 scale and the
    per-(batch,unit) bias ride free in the gelu ACTIVATE (per-partition
    scale/bias APs), so layer 1 costs nothing beyond the intrinsic gelus.
  - Layer 2 runs on the TensorEngine: per (block i, unit-pair u), bf16
    matmuls with W2-scatter stationaries accumulate eps/sig into PSUM in
    the natural partition layout.
  - Strict ACT table-set ordering: 16 gelus -> 1 exp -> erfs (3 loads).
  - erf args: p = -mu_x*inv (stt), two +/-5*step anchors, then +/-step
    chains on 2x-mode tensor_tensor.
  - diffs as strided tensor_tensor ops feeding fp16 DRAM output holding
    2*out; the 0.5 scale folds into the host dtype conversion.

Sharding: D split across 8 cores; partition p = b*4 + q holds
mu[b, q*1536:(q+1)*1536] of the core's D-shard.
"""

import sys

sys.path.insert(0, "/opt/trn_rl_repo")

import numpy as np

import concourse.bass as bass
import concourse.bacc as bacc
from concourse import mybir
from concourse.tile import TileContext
from concourse.bass_utils import run_bass_kernel_spmd

F32 = mybir.dt.float32
F16 = mybir.dt.float16
BF16 = mybir.dt.bfloat16
AF = mybir.ActivationFunctionType
OP = mybir.AluOpType

K = 16
SIGMA_ONE = 0.02
T_MIN = 1e-6
B, D, H = 32, 49152, 16
NCORES = 8
DS = D // NCORES          # 6144 columns per core
Q = 4                     # partitions per batch row
F = DS // Q               # 1536 free elements per partition
CHUNKS = [(0, 512), (512, 512), (1024, 384), (1408, 128)]   # consumption chunks (start, width)
LN_SQRT2 = 0.34657359027997264
NC_CONST = 32
# cn columns: 0..15 gelu biases (col 8*i+u), 16..23 gelu scales (col 16+u),
# 24 alpha, 25 negbeta, 26 lnA2, 27 lnm, 28 nb20, 29 -ln(sqrt 2)


def _build():
    nc = bacc.Bacc(None, target_bir_lowering=False)
    mu_p = nc.declare_dram_parameter("mu", [B, DS], F32, isOutput=False)
    cn_p = nc.declare_dram_parameter("cn", [128, NC_CONST], F32, isOutput=False)
    w2s_p = nc.declare_dram_parameter("w2s", [128, 16 * 64], BF16, isOutput=False)
    out_p = nc.declare_dram_parameter("out", [128, K * F], F16, isOutput=True)

    mu_v = mu_p.rearrange("b (q f) -> (b q) f", q=Q)

    with TileContext(nc) as tc:
        with (
            tc.tile_pool(name="const", bufs=1) as constp,
            tc.tile_pool(name="main", bufs=1) as mainp,
            tc.tile_pool(name="zs", bufs=1) as zspool,
            tc.tile_pool(name="hp", bufs=4) as hpool,
            tc.tile_pool(name="ap", bufs=1) as apool,
            tc.tile_pool(name="op", bufs=1) as opool,
            tc.tile_pool(name="ep", bufs=1, space="PSUM") as epool,
            tc.tile_pool(name="sp", bufs=1, space="PSUM") as spool,
        ):
            cn = constp.tile([128, NC_CONST], F32)
            w2s = constp.tile([128, 16 * 64], BF16)
            mu = mainp.tile([128, F], F32)
            zs_t = [zspool.tile([128, F], F32, name=f"zs{i}") for i in range(2)]
            # Split DMA issuing across sync (HWDGE) and gpsimd (SWDGE) so the
            # first gelu's inputs (cn + zs0) land as early as possible.
            zv0 = zs_t[0].rearrange("(g l) n -> g l n", l=2)
            zv1 = zs_t[1].rearrange("(g l) n -> g l n", l=2)
            nc.sync.dma_start(out=cn[:, :], in_=cn_p[:, :])
            nc.sync.dma_start(out=zv0[:, 0, 0:768], in_=mu_v[0:64, 0:768])
            nc.sync.dma_start(out=zv0[:, 1, 0:768], in_=mu_v[0:64, 0:768])
            nc.sync.dma_start(out=zv0[:, 0, 768:F], in_=mu_v[0:64, 768:F])
            nc.sync.dma_start(out=zv0[:, 1, 768:F], in_=mu_v[0:64, 768:F])
            nc.sync.dma_start(out=w2s[:, :], in_=w2s_p[:, :])
            nc.sync.dma_start(out=zv1[:, 0, :], in_=mu_v[64:128, :])
            nc.sync.dma_start(out=zv1[:, 1, :], in_=mu_v[64:128, :])
            nc.sync.dma_start(out=mu[:, :], in_=mu_v)

            # Warm the ACT engine on the const DMA semaphore while loading
            # the gelu table set.
            warm = constp.tile([128, 1], F16)
            nc.scalar.activation(out=warm, in_=cn[:, 0:1], func=AF.Gelu_apprx_tanh)

            mxn = mainp.tile([128, F], F16)
            v = mainp.tile([128, F], F16)
            inv = mainp.tile([128, F], F16)

            # -mu_x part 1 needs only mu -- run while ACT does the gelus
            # (cols 24/28 hold -alpha and +beta*b2[0] on the host side)
            nc.vector.tensor_scalar(
                out=mxn[:, :], in0=mu[:, :], scalar1=cn[:, 24:25],
                scalar2=cn[:, 28:29], op0=OP.mult, op1=OP.add)

            # ---- production: gelu(scale*zs + bias) -> MM2 contract (eps/sig)
            eps = epool.tile([128, F], F32)
            sig = spool.tile([128, F], F32)
            pend = None
            for i in range(2):
                for u in range(8):
                    h = hpool.tile([128, F], BF16)
                    # first gelu splits into column halves so ACT starts on
                    # the left half before zs0's right half finishes loading
                    parts = ((0, 768), (768, F)) if (i == 0 and u == 0) else ((0, F),)
                    for lo, hi in parts:
                        nc.scalar.activation(
                            out=h[:, lo:hi], in_=zs_t[i][:, lo:hi],
                            func=AF.Gelu_apprx_tanh,
                            scale=cn[:, 16 + u : 16 + u + 1],
                            bias=cn[:, 8 * i + u : 8 * i + u + 1],
                        )
                    if pend is not None:
                        _emit_mm2(nc, w2s, eps, sig, *pend)
                    pend = (h, i, u)
            _emit_mm2(nc, w2s, eps, sig, *pend)

            # ---- eps/sig consumption (DVE); v-C0 first -- it gates the
            # exp chain; later v chunks and exps overlap args-C0.
            sl0 = slice(CHUNKS[0][0], CHUNKS[0][0] + CHUNKS[0][1])
            nc.vector.tensor_scalar(
                out=v[:, sl0], in0=sig[:, sl0], scalar1=cn[:, 26:27],
                scalar2=cn[:, 27:28], op0=OP.add, op1=OP.max)
            nc.vector.scalar_tensor_tensor(
                out=mxn[:, :], in0=eps, scalar=cn[:, 25:26], in1=mxn[:, :],
                op0=OP.mult, op1=OP.add)
            for c0_, cw_ in CHUNKS[1:]:
                slc = slice(c0_, c0_ + cw_)
                nc.vector.tensor_scalar(
                    out=v[:, slc], in0=sig[:, slc], scalar1=cn[:, 26:27],
                    scalar2=cn[:, 27:28], op0=OP.add, op1=OP.max)

            # ---- inv = exp(-v - ln sqrt(2))  (one table switch, 3 chunks)
            for c0_, cw_ in CHUNKS:
                slc = slice(c0_, c0_ + cw_)
                nc.scalar.activation(
                    out=inv[:, slc], in_=v[:, slc], func=AF.Exp, scale=-1.0,
                    bias=cn[:, 29:30])

            # ---- consumption: args (DVE) / erf (ACT) / diffs+stores (DVE)
            # DVE order: argsC0, argsC1, diffsC0, argsC2, diffsC1, diffsC2
            # ACT order: erfC0, erfC1, erfC2 (table already on erf set)
            a_t = {}

            def emit_args(c):
                c0, cw = CHUNKS[c]
                sl = slice(c0, c0 + cw)
                a = apool.tile([128, 15, cw], F16, name=f"a{c}", tag=f"a{c}")
                a_t[c] = a
                s1 = apool.tile([128, cw], F16, name=f"s1_{c}", tag=f"s1_{c}")
                s5 = apool.tile([128, cw], F16, name=f"s5_{c}", tag=f"s5_{c}")
                pt = apool.tile([128, cw], F16, name=f"pt_{c}", tag=f"pt_{c}")

                def step(src_, dst):
                    in0 = pt if src_ == 7 else a[:, src_, :]
                    nc.vector.tensor_tensor(
                        out=a[:, dst, :], in0=in0, in1=s1,
                        op=OP.add if dst > src_ else OP.subtract)

                # erf ops are interleaved with the arg chains, ordered by
                # argument readiness so ACT starts earlier. erf runs in
                # place, so the shared anchor p lives in its own tile (pt);
                # anything read after an erf of its slice must come from pt.
                nc.vector.tensor_scalar_mul(out=s1, in0=inv[:, sl], scalar1=0.125)
                nc.vector.tensor_tensor(
                    out=pt, in0=mxn[:, sl], in1=inv[:, sl], op=OP.mult)
                nc.vector.tensor_copy(a[:, 7, :], pt)
                step(7, 6)
                step(6, 5)
                nc.scalar.activation(out=a[:, 5:8, :], in_=a[:, 5:8, :], func=AF.Erf)
                step(7, 8)
                step(8, 9)
                nc.scalar.activation(out=a[:, 8:10, :], in_=a[:, 8:10, :], func=AF.Erf)
                nc.vector.tensor_scalar_mul(out=s5, in0=inv[:, sl], scalar1=0.625)
                nc.vector.tensor_tensor(
                    out=a[:, 2, :], in0=pt, in1=s5, op=OP.subtract)
                step(2, 1)
                step(1, 0)
                step(2, 3)
                step(3, 4)
                nc.scalar.activation(out=a[:, 0:5, :], in_=a[:, 0:5, :], func=AF.Erf)
                nc.vector.tensor_tensor(
                    out=a[:, 12, :], in0=pt, in1=s5, op=OP.add)
                step(12, 11)
                step(11, 10)
                step(12, 13)
                step(13, 14)
                nc.scalar.activation(out=a[:, 10:15, :], in_=a[:, 10:15, :], func=AF.Erf)

            def emit_diffs(c):
                c0, cw = CHUNKS[c]
                # last chunk's stores ride the scalar engine's free HWDGE
                # queue, in parallel with sync's transfer backlog
                eng = nc.scalar if c == len(CHUNKS) - 1 else nc.sync
                a = a_t[c]
                o = opool.tile([128, K, cw], F16, name=f"o{c}", tag=f"o{c}")
                nc.vector.tensor_scalar_add(out=o[:, 0, :], in0=a[:, 0, :], scalar1=1.0)
                nc.vector.tensor_tensor(
                    out=o[:, 1:4, :], in0=a[:, 1:4, :], in1=a[:, 0:3, :],
                    op=OP.subtract)
                nc.vector.tensor_tensor(
                    out=o[:, 4:8, :], in0=a[:, 4:8, :], in1=a[:, 3:7, :],
                    op=OP.subtract)
                eng.dma_start(
                    out=out_p[:, K * c0 : K * c0 + 8 * cw],
                    in_=o[:, 0:8, :].rearrange("p k n -> p (k n)"))
                nc.vector.tensor_tensor(
                    out=o[:, 8:12, :], in0=a[:, 8:12, :], in1=a[:, 7:11, :],
                    op=OP.subtract)
                nc.vector.tensor_scalar(
                    out=o[:, 15, :], in0=a[:, 14, :], scalar1=-1.0, scalar2=1.0,
                    op0=OP.mult, op1=OP.add)
                nc.vector.tensor_tensor(
                    out=o[:, 12:15, :], in0=a[:, 12:15, :], in1=a[:, 11:14, :],
                    op=OP.subtract)
                eng.dma_start(
                    out=out_p[:, K * c0 + 8 * cw : K * c0 + 16 * cw],
                    in_=o[:, 8:16, :].rearrange("p k n -> p (k n)"))

            emit_args(0)
            emit_args(1)
            emit_diffs(0)
            emit_args(2)
            emit_diffs(1)
            emit_args(3)
            emit_diffs(2)
            emit_diffs(3)

    return nc


def _emit_mm2(nc, w2s, eps, sig, h, i, u):
    st = u == 0
    sp = u == 7
    for col, dst in ((1, sig), (0, eps)):
        lhs2 = w2s[:, (2 * u + col) * 64 : (2 * u + col) * 64 + 64]
        for c in range(3):
            nc.tensor.matmul(
                dst[64 * i : 64 * i + 64, 512 * c : 512 * (c + 1)], lhs2,
                h[:, 512 * c : 512 * (c + 1)], start=st, stop=sp)


def _host_consts(t, W1, b1, W2, b2):
    t64 = np.asarray(t, np.float64).reshape(B)
    cond = t64 < T_MIN
    gamma = 1.0 - SIGMA_ONE ** (2.0 * t64)
    alpha = np.where(cond, 0.0, 1.0 / np.where(gamma == 0, 1.0, gamma))
    beta = np.sqrt(np.maximum(1.0 - gamma, 0.0) / np.where(gamma == 0, 1.0, gamma))
    negbeta = np.where(cond, 0.0, -beta)
    lnA2 = np.where(cond, -1e4, np.log(np.maximum(beta, 1e-300)) + float(b2[1]))
    lnm = np.where(cond, 0.0, np.log(SIGMA_ONE))
    nb20 = np.where(cond, 0.0, -beta * float(b2[0]))

    cn = np.zeros((128, NC_CONST), np.float32)
    # gelu biases/scales at expanded layout: partition p = 2*g + l handles
    # source row s = 64*i + g (batch b = s//4) and unit j = 2*u + l.
    cvals = (t64[:, None] * np.asarray(W1, np.float64)[1, :][None, :]
             + np.asarray(b1, np.float64)[None, :])          # [B, H]
    for u in range(8):
        for g in range(64):
            for l in range(2):
                p = 2 * g + l
                cn[p, 16 + u] = W1[0, 2 * u + l]
                for i in range(2):
                    cn[p, 8 * i + u] = cvals[16 * i + g // 4, 2 * u + l]
    # mxn = -mu_x accumulates as (-alpha)*mu + beta*b2[0] + beta*eps
    for p in range(128):
        bb = p // Q
        cn[p, 24] = -alpha[bb]
        cn[p, 25] = -negbeta[bb]
        cn[p, 26] = lnA2[bb]
        cn[p, 27] = lnm[bb]
        cn[p, 28] = -nb20[bb]
        cn[p, 29] = -LN_SQRT2

    # MM2 stationary: w2s[2*g+l, (2u+col)*64 + m] = (g == m) * W2[2u+l, col]
    w2s = np.zeros((128, 16 * 64), np.float32)
    for u in range(8):
        for col in range(2):
            s0 = (2 * u + col) * 64
            for m in range(64):
                for l in range(2):
                    w2s[2 * m + l, s0 + m] = W2[2 * u + l, col]
    import ml_dtypes
    w2s = w2s.astype(ml_dtypes.bfloat16)

    return cn, w2s


def _run(inputs, trace=False):
    mu = np.ascontiguousarray(np.asarray(inputs["mu"], np.float32))
    t = np.asarray(inputs["t"], np.float32)
    W1 = np.asarray(inputs["W1"], np.float32)
    b1 = np.asarray(inputs["b1"], np.float32)
    W2 = np.asarray(inputs["W2"], np.float32)
    b2 = np.asarray(inputs["b2"], np.float32)

    nc = _build()
    nc.finalize()
    cn, w2s = _host_consts(t, W1, b1, W2, b2)

    in_maps = []
    for c in range(NCORES):
        shard = np.ascontiguousarray(mu[:, c * DS : (c + 1) * DS])
        in_maps.append({"mu": shard, "cn": cn, "w2s": w2s})

    res = run_bass_kernel_spmd(nc, in_maps, list(range(NCORES)), trace=trace)
    shards = []
    for c in range(NCORES):
        s = np.asarray(res.results[c]["out"])          # [128, K*F] f16 (2*out)
        blocks = []
        for c0, cw in CHUNKS:
            blocks.append(s[:, K * c0 : K * (c0 + cw)].reshape(128, K, cw))
        s = np.concatenate(blocks, axis=2)             # [128, K, F]
        s = s.reshape(B, Q, K, F).transpose(0, 1, 3, 2).reshape(B, DS, K)
        shards.append(s)
    out = np.concatenate(shards, axis=1).astype(np.float32)
    out *= np.float32(0.5)
    return np.ascontiguousarray(out), res


def kernel(**inputs) -> np.ndarray:
    out, _ = _run(inputs, trace=False)
    return out


if __name__ == "__main__":
    rng = np.random.default_rng(0)
    demo = {
        "mu": rng.standard_normal((B, D), dtype=np.float32),
        "t": rng.random((B, 1), dtype=np.float32),
        "W1": rng.standard_normal((2, H), dtype=np.float32) * 0.5,
        "b1": rng.standard_normal((H,), dtype=np.float32) * 0.1,
        "W2": rng.standard_normal((H, 2), dtype=np.float32) * 0.1,
        "b2": rng.standard_normal((2,), dtype=np.float32) * 0.1,
    }
    out = kernel(**demo)
    print("kernel output", out.shape, out.dtype, out[0, 0])
